# revision 30
# baseline (speedup 1.0000x reference)
"""Differential attention + quirky GroupNorm + output proj on 8 TRN2 NeuronCores.

Strategy (head-sharded attention, token-sharded norm+proj):
  - Host preps transposed layouts (xT, per-core wqkvT slice, woT), RoPE
    cos/sin tables, band masks, and selector constants with -lambda baked in
    (lambda is a host-computable scalar). External inputs stay fp32 (bf16 I/O
    corrupts via PJRT); weights/activations convert to bf16 on device so all
    big matmuls run 1-pass bf16 with FWL weight loads.
  - Each core computes QKV for its 2 heads (full-E contraction), applies
    RoPE, and runs the two differential softmaxes (full S x S, float +1.0
    mask above the diagonal - NOT causal). The two 64-dim score
    contractions are issued into one [128,1024] PSUM pair as row-disjoint
    concurrent matmuls (array rows 0-63 / 64-127), so ONE exp per kt
    evacuates both halves (halving ACT's 352-cycle per-instruction cost).
    Sum rows and PV accumulate inline in the kt loop.
  - Softmax normalization is deferred past the AllToAll: cores exchange
    unnormalized U1,U2 (bf16) plus the two sum rows per head (f32). After
    the A2A each core normalizes ITS 512 tokens: one batched approx
    reciprocal over [32,512], a K=32 selector matmul broadcasts each
    (head,half) row over 128 channels, and two tensor ops form
    Ahat = U1/s1 - lambda*U2/s2 per head.
  - Post-collective loads ride the GpSimd (SWDGE) DMA queue, which already
    sits behind the collectives, so they cannot head-of-line-block the
    Sync queue that feeds the second head's attention.
  - GroupNorm groups = 128-token blocks x all channels (core-local).
    out[t,:] = 0.2*(gamma[t]*Ahat[t,:] + beta[t]) @ woT with the norm scale
    folded into PSUM evacuation and beta/mean terms in one rank-1 row term.
"""

import math

import numpy as np

import concourse.bass as bass
import concourse.tile as tile
from concourse import bacc, mybir
from concourse.bass_utils import run_bass_kernel_spmd

F32 = mybir.dt.float32
F32R = mybir.dt.float32r
BF16 = mybir.dt.bfloat16
AX = mybir.AxisListType
OP = mybir.AluOpType
ACTF = mybir.ActivationFunctionType

B, S, E, H = 2, 2048, 2048, 16
HD = E // H                # 128
NC = 8                     # cores
HPC = H // NC              # 2 heads per core
CPC = HPC * HD             # 256 channels per core
T = B * S                  # 4096 tokens
TPC = T // NC              # 512 tokens per core
NG = TPC // HD             # 4 groups per core (128-token blocks)
GROUP_N = float(HD * E)    # 262144 elements per group
INIT_LAMBDA = 0.8
EPS = 1e-5
SCALER = HD ** -0.5


def _mm(x):
    return x.bitcast(F32R)


def _attn_bh(nc, pools, consts, h, b):
    """Differential attention for one (batch, local-head) pair."""
    vp, pp, s2tmp, ps_sc, ps_sum, ps_o = pools
    q_sb, k_sb, masks_sb, ones_col_bf, v_dram, a2a_in = consts
    bh = b * HPC + h
    v_bh = vp.tile([128, 16 * HD], BF16, tag="v_bh")
    nc.sync.dma_start(
        v_bh[:].rearrange("p (kt d) -> p kt d", kt=16),
        v_dram[b, :, h * HD:(h + 1) * HD].rearrange("(kt p) d -> p kt d", p=128),
    )
    for qc in range(4):
        sum0 = ps_sum.tile([1, 512], F32, tag="sum0")
        sum1 = ps_sum.tile([1, 512], F32, tag="sum1")
        sums = [sum0, sum1]
        op0 = ps_o.tile([128, 512], F32, tag="o0")
        op1 = ps_o.tile([128, 512], F32, tag="o1")
        ops = [op0, op1]
        for kt in range(16):
            # both halves' scores into one 2-bank PSUM tile; the K=64
            # matmuls target disjoint row groups (0-63 / 64-127) and run
            # concurrently on the PE array.
            sc = ps_sc.tile([128, 1024], F32, tag="sc")
            for half in range(2):
                hs = half * 64
                nc.tensor.matmul(
                    sc[:, half * 512:(half + 1) * 512],
                    k_sb[hs:hs + 64,
                         bh * S + kt * 128:bh * S + kt * 128 + 128],
                    q_sb[hs:hs + 64,
                         bh * S + qc * 512:bh * S + (qc + 1) * 512],
                    start=True, stop=True,
                )
            o = kt - 4 * qc
            pt = pp.tile([128, 1024], BF16, tag="p")
            if 0 <= o < 4:
                scr = s2tmp.tile([128, 1024], F32, tag="bandscr")
                nc.vector.scalar_tensor_tensor(
                    out=scr[:].rearrange("p (two n) -> p two n", two=2),
                    in0=sc[:].rearrange("p (two n) -> p two n", two=2),
                    scalar=SCALER,
                    in1=masks_sb[:, o * 512:(o + 1) * 512]
                    .rearrange("p (one n) -> p one n", one=1)
                    .to_broadcast([128, 2, 512]),
                    op0=OP.mult, op1=OP.add,
                )
                nc.scalar.activation(pt[:], scr[:], ACTF.Exp)
            else:
                bias = 1.0 if o >= 4 else 0.0
                nc.scalar.activation(pt[:], sc[:], ACTF.Exp,
                                     bias=bias, scale=SCALER)
            for half in range(2):
                psl = pt[:, half * 512:(half + 1) * 512]
                nc.tensor.matmul(
                    sums[half][:], ones_col_bf[:], psl,
                    start=(kt == 0), stop=(kt == 15),
                )
                nc.tensor.matmul(
                    ops[half][:], v_bh[:, kt * HD:(kt + 1) * HD], psl,
                    start=(kt == 0), stop=(kt == 15),
                )
        # evacuate unnormalized U1,U2 + sum rows (all bf16) to the A2A
        dest = b * 4 + qc
        for half in range(2):
            ev = s2tmp.tile([128, 512], BF16, tag=f"ev{half}")
            nc.vector.tensor_copy(ev[:], ops[half][:])
            nc.sync.dma_start(
                a2a_in[h][dest, half * HD:(half + 1) * HD, :], ev[:])
            s_ev = s2tmp.tile([1, 512], BF16, tag=f"s_ev{half}")
            nc.vector.tensor_copy(s_ev[:], sums[half][:])
            nc.sync.dma_start(a2a_in[h][dest, 2 * HD + half, :], s_ev[0, :])


def _stage4(nc, tc, pools, consts):
    """Post-A2A: normalize+combine per head, GroupNorm stats, projection."""
    (s4res, up, s4tmp, sqp, wstg4, wop,
     ps_bc, ps_st, ps_rs, ps_b, ps_p) = pools
    (a2a_out, sel, gb, woT, out,
     ones_col, ones_row, ones_col_bf) = consts

    # sums for my 512 tokens: [32,512], partition p = hl*16 + src*2 + half
    s_bf = s4res.tile([32, 512], BF16, tag="s_bf")
    for hl in range(HPC):
        nc.sync.dma_start(
            s_bf[hl * 16:(hl + 1) * 16, :]
            .rearrange("(s two) t -> s two t", s=NC, two=2),
            a2a_out[hl][:, 2 * HD:2 * HD + 2, :],
        )
    s_sb = s4res.tile([32, 512], F32, tag="s_sb")
    nc.vector.tensor_copy(s_sb[:], s_bf[:])
    s_rec = s4res.tile([32, 512], F32, tag="s_rec")
    nc.vector.reciprocal_approx_fast(out=s_rec[:], in_=s_sb[:])
    rbf = s4res.tile([32, 512], BF16, tag="rbf")
    nc.vector.tensor_copy(rbf[:], s_rec[:])

    sel_st = s4res.tile([32, 32 * 128], F32, tag="sel_st")
    nc.sync.dma_start(sel_st[:].bitcast(F32R), sel[:, :].bitcast(F32R))
    sel_bf = s4res.tile([32, 32 * 128], BF16, tag="sel_bf")
    nc.vector.tensor_copy(sel_bf[:], sel_st[:])

    g_sb = s4res.tile([128, 16 * TPC], BF16, tag="g_sb")
    st_sum = s4res.tile([128, NG, 16], F32, tag="st_sum")
    st_sq = s4res.tile([128, NG * 16], F32, tag="st_sq")
    for ct in range(16):
        src, hl = ct // 2, ct % 2
        u1 = up.tile([128, 512], BF16, tag="u1")
        nc.sync.dma_start(u1[:], a2a_out[hl][src, 0:HD, :])
        u2 = up.tile([128, 512], BF16, tag="u2")
        nc.sync.dma_start(u2[:], a2a_out[hl][src, HD:2 * HD, :])
        th = [None, None]
        for half in range(2):
            p = hl * 16 + src * 2 + half
            bc_ps = ps_bc.tile([128, 512], F32, tag=f"bc{half}")
            nc.tensor.matmul(
                bc_ps[:], sel_bf[:, p * 128:(p + 1) * 128], rbf[:],
                start=True, stop=True,
            )
            t = s4tmp.tile([128, 512], F32, tag=f"th{half}")
            nc.vector.tensor_tensor(t[:], [u1, u2][half][:], bc_ps[:], OP.mult)
            th[half] = t
        gt = g_sb[:, ct * TPC:(ct + 1) * TPC]
        nc.vector.tensor_tensor(gt, th[0][:], th[1][:], OP.add)
        nc.vector.tensor_reduce(
            st_sum[:, :, ct],
            gt.rearrange("p (g n) -> p g n", g=NG),
            AX.X, OP.add,
        )
        for g in range(NG):
            scr = sqp.tile([128, HD], F32, tag="sqs")
            nc.scalar.activation(
                scr[:], gt[:, g * HD:(g + 1) * HD], ACTF.Square,
                accum_out=st_sq[:, g * 16 + ct:g * 16 + ct + 1],
            )

    # ---- stats finalize ----
    red8 = s4res.tile([128, 2 * NG], F32, tag="red8")
    nc.vector.tensor_reduce(
        red8[:, 0:NG].bitcast(F32R), st_sum[:], AX.X, OP.add
    )
    nc.vector.tensor_reduce(
        red8[:, NG:2 * NG].bitcast(F32R),
        st_sq[:].rearrange("p (g c) -> p g c", g=NG),
        AX.X, OP.add
    )
    stat_ps = ps_st.tile([1, 2 * NG], F32, tag="statp")
    nc.tensor.matmul(stat_ps[:], _mm(ones_col[:]), _mm(red8[:]),
                     start=True, stop=True)
    srow = s4res.tile([1, 2 * NG], F32, tag="srow")
    nc.vector.tensor_scalar(
        out=srow[:], in0=stat_ps[:], scalar1=1.0 / GROUP_N,
        scalar2=None, op0=OP.mult,
    )  # [mean_g | E[x^2]_g]
    var_r = s4res.tile([1, NG], F32, tag="var_r")
    m2 = s4res.tile([1, NG], F32, tag="m2")
    nc.vector.tensor_tensor(m2[:], srow[:, 0:NG], srow[:, 0:NG], OP.mult)
    nc.vector.tensor_tensor(var_r[:], srow[:, NG:2 * NG], m2[:], OP.subtract)
    eps_t = s4res.tile([1, 1], F32, tag="eps_t")
    nc.gpsimd.memset(eps_t[:], EPS)
    std_r = s4res.tile([1, NG], F32, tag="std_r")
    nc.scalar.activation(std_r[:], var_r[:], ACTF.Sqrt, bias=eps_t[:])
    ab_row = s4res.tile([1, 2 * NG], F32, tag="ab_row")
    nc.vector.reciprocal(ab_row[:, 0:NG].bitcast(F32R), std_r[:])
    mtmp = s4res.tile([1, NG], F32, tag="mtmp")
    nc.vector.tensor_tensor(mtmp[:], srow[:, 0:NG], ab_row[:, 0:NG], OP.mult)
    nc.vector.tensor_scalar(
        out=ab_row[:, NG:2 * NG].bitcast(F32R), in0=mtmp[:],
        scalar1=-1.0, scalar2=None, op0=OP.mult,
    )  # b_g = -mean*rstd
    ab_ps = ps_st.tile([128, 2 * NG], F32, tag="abp")
    nc.tensor.matmul(ab_ps[:], _mm(ones_row[:]), _mm(ab_row[:]),
                     start=True, stop=True)
    ab_bc = s4res.tile([128, 2 * NG], F32, tag="ab_bc")
    nc.scalar.copy(ab_bc[:], ab_ps[:])

    gamma_col = s4res.tile([128, NG], F32, tag="gamma_col")
    nc.sync.dma_start(
        gamma_col[:], gb[0, :].rearrange("(c p) -> p c", p=128)
    )
    m1_col = s4res.tile([128, NG], F32, tag="m1_col")
    nc.vector.scalar_tensor_tensor(
        out=m1_col[:], in0=gamma_col[:], scalar=(1.0 - INIT_LAMBDA),
        in1=ab_bc[:, 0:NG], op0=OP.mult, op1=OP.mult,
    )
    gamma_row = s4res.tile([1, TPC], F32, tag="gamma_row")
    nc.sync.dma_start(gamma_row[:], gb[0:1, :])
    beta_row = s4res.tile([1, TPC], F32, tag="beta_row")
    nc.sync.dma_start(beta_row[:], gb[1:2, :])
    m2_row = s4res.tile([1, TPC], F32, tag="m2_row")
    m2tmp = s4res.tile([1, TPC], F32, tag="m2tmp")
    nc.vector.tensor_tensor(
        m2tmp[:].rearrange("o (g u) -> o g u", g=NG),
        gamma_row[:].rearrange("o (g u) -> o g u", g=NG),
        ab_row[:, NG:2 * NG].rearrange("o (g u) -> o g u", u=1)
        .to_broadcast([1, NG, HD]),
        OP.mult,
    )
    nc.vector.tensor_tensor(m2tmp[:], m2tmp[:], beta_row[:], OP.add)
    nc.vector.tensor_scalar(
        out=m2_row[:].bitcast(F32R), in0=m2tmp[:],
        scalar1=(1.0 - INIT_LAMBDA), scalar2=None, op0=OP.mult,
    )

    # ---- output projection ----
    rowsum = s4res.tile([1, E], F32, tag="rowsum")
    for oc in range(4):
        wots = []
        for ct in range(16):
            wst = wstg4.tile([128, 512], F32, tag="wst4")
            nc.sync.dma_start(
                wst[:].bitcast(F32R),
                woT[ct * 128:(ct + 1) * 128,
                    oc * 512:(oc + 1) * 512].bitcast(F32R),
            )
            wot = wop.tile([128, 512], BF16, tag="wot")
            nc.vector.tensor_copy(wot[:], wst[:])
            wots.append(wot)
        rs_ps = ps_rs.tile([1, 512], F32, tag="rs")
        for ct in range(16):
            nc.tensor.matmul(rs_ps[:], ones_col_bf[:], wots[ct][:],
                             start=(ct == 0), stop=(ct == 15))
        nc.scalar.copy(rowsum[:, oc * 512:(oc + 1) * 512].bitcast(F32R),
                       rs_ps[:])
        for tcg in range(NG):
            po = ps_p.tile([128, 512], F32, tag="po")
            for ct in range(16):
                nc.tensor.matmul(
                    po[:],
                    g_sb[:, ct * TPC + tcg * 128:ct * TPC + (tcg + 1) * 128],
                    wots[ct][:],
                    start=(ct == 0), stop=(ct == 15),
                )
            bps = ps_b.tile([128, 512], F32, tag="bps")
            nc.tensor.matmul(
                bps[:], _mm(m2_row[0:1, tcg * 128:(tcg + 1) * 128]),
                _mm(rowsum[0:1, oc * 512:(oc + 1) * 512]),
                start=True, stop=True,
            )
            osb = s4tmp.tile([128, 512], F32, tag="osb")
            nc.scalar.activation(osb[:], po[:], ACTF.Copy,
                                 scale=m1_col[:, tcg:tcg + 1])
            nc.vector.tensor_tensor(osb[:], osb[:], bps[:], OP.add)
            nc.sync.dma_start(
                out[tcg * 128:(tcg + 1) * 128, oc * 512:(oc + 1) * 512],
                osb[:],
            )


def build_nc():
    nc = bacc.Bacc("TRN2", target_bir_lowering=False, debug=False, num_devices=NC)

    xT = nc.declare_dram_parameter("xT", [E, T], F32, isOutput=False)
    wqkvT = nc.declare_dram_parameter("wqkvT", [E, 3 * CPC], F32, isOutput=False)
    woT = nc.declare_dram_parameter("woT", [E, E], F32, isOutput=False)
    cosd = nc.declare_dram_parameter("cosd", [HD, S], F32, isOutput=False)
    sind = nc.declare_dram_parameter("sind", [HD, S], F32, isOutput=False)
    bandm = nc.declare_dram_parameter("bandm", [4, 128, 512], F32, isOutput=False)
    sel = nc.declare_dram_parameter("sel", [32, 32 * 128], F32, isOutput=False)
    ones128 = nc.declare_dram_parameter("ones128", [1, 128], F32, isOutput=False)
    gb = nc.declare_dram_parameter("gb", [2, TPC], F32, isOutput=False)
    out = nc.declare_dram_parameter("out", [TPC, E], F32, isOutput=True)

    # internal DRAM
    v_dram = nc.dram_tensor("v_dram", [B, S, CPC], BF16)
    a2a_in = [nc.dram_tensor(f"a2a_in{h}", [NC, 260, TPC], BF16)
              for h in range(HPC)]
    a2a_out = [nc.dram_tensor(f"a2a_out{h}", [NC, 260, TPC], BF16)
               for h in range(HPC)]

    with tile.TileContext(nc) as tc, \
         nc.allow_low_precision(reason="bf16 matmuls; tolerance 2e-2"):
        with tc.tile_pool(name="small", bufs=1) as small:
            ones_col = small.tile([128, 1], F32, tag="ones_col")
            nc.sync.dma_start(
                ones_col[:].bitcast(F32R),
                ones128[0, :].rearrange("(p o) -> p o", o=1).bitcast(F32R),
            )
            ones_row = small.tile([1, 128], F32, tag="ones_row")
            nc.sync.dma_start(ones_row[:].bitcast(F32R),
                              ones128[:, :].bitcast(F32R))
            ones_col_bf = small.tile([128, 1], BF16, tag="ones_col_bf")
            nc.vector.tensor_copy(ones_col_bf[:], ones_col[:])

            with tc.tile_pool(name="qkres", bufs=1) as qkres:
                q_sb = qkres.tile([128, 2 * HPC * S], BF16, tag="q_sb")
                k_sb = qkres.tile([128, 2 * HPC * S], BF16, tag="k_sb")

                # ======== stage 1: QKV + RoPE (bf16 matmuls) ========
                with tc.tile_pool(name="s1res", bufs=1) as s1res, \
                     tc.tile_pool(name="wstg", bufs=3) as wstg, \
                     tc.tile_pool(name="xtp", bufs=6) as xtp, \
                     tc.tile_pool(name="xbfp", bufs=20) as xbfp, \
                     tc.tile_pool(name="s1tmp", bufs=6) as s1tmp, \
                     tc.tile_pool(name="vtmp", bufs=4) as vtmp, \
                     tc.tile_pool(name="ps_qk", bufs=5, space="PSUM") as ps_qk, \
                     tc.tile_pool(name="ps_v", bufs=3, space="PSUM") as ps_v:
                    w_sb = s1res.tile([128, 16 * 3 * CPC], BF16, tag="w_sb")
                    for et in range(16):
                        wst = wstg.tile([128, 3 * CPC], F32, tag="wst")
                        nc.sync.dma_start(
                            wst[:].bitcast(F32R),
                            wqkvT[et * 128:(et + 1) * 128, :].bitcast(F32R),
                        )
                        nc.vector.tensor_copy(
                            w_sb[:, et * 3 * CPC:(et + 1) * 3 * CPC], wst[:]
                        )
                    cos_sb = s1res.tile([HD, S], F32, tag="cos_sb")
                    nc.sync.dma_start(cos_sb[:], cosd[:, :])
                    sin_sb = s1res.tile([HD, S], F32, tag="sin_sb")
                    nc.sync.dma_start(sin_sb[:], sind[:, :])

                    for tci in range(T // 512):
                        b = tci // 4
                        sc = tci % 4
                        xbfs = []
                        for et in range(16):
                            xt = xtp.tile([128, 512], F32, tag="xt")
                            nc.sync.dma_start(
                                xt[:].bitcast(F32R),
                                xT[et * 128:(et + 1) * 128,
                                   tci * 512:(tci + 1) * 512].bitcast(F32R),
                            )
                            xbf = xbfp.tile([128, 512], BF16, tag="xbf")
                            if et % 2 == 0:
                                nc.vector.tensor_copy(xbf[:], xt[:])
                            else:
                                nc.scalar.copy(xbf[:], xt[:])
                            xbfs.append(xbf)
                        # q, k channel-major [hd, 512 tokens] per local head
                        for which, dst in ((0, q_sb), (1, k_sb)):
                            for h in range(HPC):
                                ps = ps_qk.tile([128, 512], F32, tag="psqk")
                                for et in range(16):
                                    wcol = et * 3 * CPC + which * CPC + h * HD
                                    nc.tensor.matmul(
                                        ps[:],
                                        w_sb[:, wcol:wcol + HD],
                                        xbfs[et][:],
                                        start=(et == 0), stop=(et == 15),
                                    )
                                # RoPE: dst = ps*cos + rotperm(ps)*sinsgn
                                csl = cos_sb[:, sc * 512:(sc + 1) * 512]
                                ssl = sin_sb[:, sc * 512:(sc + 1) * 512]
                                qc = s1tmp.tile([128, 512], F32, tag="ropeqc")
                                nc.vector.tensor_tensor(qc[:], ps[:], csl, OP.mult)
                                rot = s1tmp.tile([128, 512], F32, tag="roperot")
                                nc.scalar.copy(rot[0:64, :], ps[64:128, :])
                                nc.scalar.copy(rot[64:128, :], ps[0:64, :])
                                nc.vector.tensor_tensor(rot[:], rot[:], ssl, OP.mult)
                                col = (b * HPC + h) * S + sc * 512
                                nc.vector.tensor_tensor(
                                    dst[:, col:col + 512], qc[:], rot[:], OP.add
                                )
                        # v token-major [t, 256]
                        for ts4 in range(4):
                            ps = ps_v.tile([128, CPC], F32, tag="psv")
                            for et in range(16):
                                wcol = et * 3 * CPC + 2 * CPC
                                nc.tensor.matmul(
                                    ps[:],
                                    xbfs[et][:, ts4 * 128:(ts4 + 1) * 128],
                                    w_sb[:, wcol:wcol + CPC],
                                    start=(et == 0), stop=(et == 15),
                                )
                            vsb = vtmp.tile([128, CPC], BF16, tag="vsb")
                            nc.scalar.copy(vsb[:], ps[:])
                            trow = sc * 512 + ts4 * 128
                            nc.sync.dma_start(
                                v_dram[b, trow:trow + 128, :], vsb[:]
                            )

                # ======== stage 2: differential attention ========
                with tc.tile_pool(name="s2res", bufs=1) as s2res, \
                     tc.tile_pool(name="vp", bufs=2) as vp, \
                     tc.tile_pool(name="pp", bufs=6) as pp, \
                     tc.tile_pool(name="s2tmp", bufs=2) as s2tmp, \
                     tc.tile_pool(name="s4res", bufs=1) as s4res, \
                     tc.tile_pool(name="up", bufs=4) as up, \
                     tc.tile_pool(name="wstg4", bufs=2) as wstg4, \
                     tc.tile_pool(name="wop", bufs=16) as wop, \
                     tc.tile_pool(name="s4tmp", bufs=4) as s4tmp, \
                     tc.tile_pool(name="sqscratch", bufs=2) as sqp:
                    with tc.tile_pool(name="ps_sc", bufs=2, space="PSUM") as ps_sc, \
                         tc.tile_pool(name="ps_sum", bufs=1, space="PSUM") as ps_sum, \
                         tc.tile_pool(name="ps_o", bufs=1, space="PSUM") as ps_o:
                        masks_sb = s2res.tile([128, 4 * 512], F32, tag="masks_sb")
                        for o in range(4):
                            nc.sync.dma_start(
                                masks_sb[:, o * 512:(o + 1) * 512], bandm[o, :, :]
                            )
                        pools = (vp, pp, s2tmp, ps_sc, ps_sum, ps_o)
                        consts = (q_sb, k_sb, masks_sb, ones_col_bf,
                                  v_dram, a2a_in)
                        for h in range(HPC):
                            for b in range(B):
                                _attn_bh(nc, pools, consts, h, b)
                            nc.gpsimd.collective_compute(
                                "AllToAll",
                                OP.bypass,
                                replica_groups=[list(range(NC))],
                                ins=[a2a_in[h].ap().opt()],
                                outs=[a2a_out[h].ap().opt()],
                            )

                    # ======== stage 4 ========
                    with tc.tile_pool(name="ps_bc", bufs=1, space="PSUM") as ps_bc, \
                         tc.tile_pool(name="ps_st", bufs=1, space="PSUM") as ps_st, \
                         tc.tile_pool(name="ps_rs", bufs=1, space="PSUM") as ps_rs, \
                         tc.tile_pool(name="ps_b", bufs=1, space="PSUM") as ps_b, \
                         tc.tile_pool(name="ps_p", bufs=2, space="PSUM") as ps_p:
                        pools4 = (s4res, up, s4tmp, sqp, wstg4, wop,
                                  ps_bc, ps_st, ps_rs, ps_b, ps_p)
                        consts4 = (a2a_out, sel, gb, woT, out,
                                   ones_col, ones_row, ones_col_bf)
                        _stage4(nc, tc, pools4, consts4)

    nc.compile()
    return nc


_NC_CACHE = None


def _get_nc():
    global _NC_CACHE
    if _NC_CACHE is None:
        _NC_CACHE = build_nc()
    return _NC_CACHE


def _host_prep(x, w_qkv, wo, lambda_q1, lambda_q2, lambda_k1, lambda_k2,
               gamma, beta):
    x = np.asarray(x, dtype=np.float32)
    w_qkv = np.asarray(w_qkv, dtype=np.float32)
    wo = np.asarray(wo, dtype=np.float32)
    gamma = np.asarray(gamma, dtype=np.float32)
    beta = np.asarray(beta, dtype=np.float32)

    xT = np.ascontiguousarray(x.reshape(T, E).T)
    woT = np.ascontiguousarray(wo.T)

    # RoPE tables, channel-major with sign folded into sin
    inv = 1.0 / (10000.0 ** (np.arange(0, HD, 2, dtype=np.float32) / HD))
    ang = np.arange(S, dtype=np.float32)[:, None] * inv[None, :]  # (S, 64)
    ang = np.concatenate([ang, ang], axis=-1)                     # (S, 128)
    cosd = np.ascontiguousarray(np.cos(ang).T.astype(np.float32))  # (128, S)
    sin_t = np.sin(ang).T.astype(np.float32)
    sind = np.ascontiguousarray(
        np.concatenate([-sin_t[:64], sin_t[64:]], axis=0)
    )

    # band masks: mask_o[ki, qi] = 1.0 iff (o*128 + ki) > qi
    o_idx = np.arange(4)[:, None, None] * 128
    ki = np.arange(128)[None, :, None]
    qi = np.arange(512)[None, None, :]
    bandm = ((o_idx + ki) > qi).astype(np.float32)

    # lambda scalar computed on host; -lambda baked into selector rows 16-31
    lam = float(np.exp(np.sum(lambda_q1 * lambda_k1))
                - np.exp(np.sum(lambda_q2 * lambda_k2)) + INIT_LAMBDA)
    # selector row p corresponds to (src, h, half) with p = src*4 + h*2 + half
    sel = np.zeros((32, 32 * 128), dtype=np.float32)
    for p in range(32):
        val = 1.0 if p % 2 == 0 else -lam
        sel[p, p * 128:(p + 1) * 128] = val

    in_maps = []
    for j in range(NC):
        h0 = HPC * j
        rows_q = w_qkv[h0 * HD:(h0 + HPC) * HD, :]
        rows_k = w_qkv[E + h0 * HD:E + (h0 + HPC) * HD, :]
        rows_v = w_qkv[2 * E + h0 * HD:2 * E + (h0 + HPC) * HD, :]
        wqkvT = np.ascontiguousarray(
            np.concatenate([rows_q.T, rows_k.T, rows_v.T], axis=1)
            .astype(np.float32)
        )
        sl = (j % 4) * TPC
        gbj = np.ascontiguousarray(
            np.stack([gamma[sl:sl + TPC], beta[sl:sl + TPC]])
        )
        in_maps.append({
            "ones128": np.ones((1, 128), dtype=np.float32),
            "xT": xT,
            "wqkvT": wqkvT,
            "woT": woT,
            "cosd": cosd,
            "sind": sind,
            "bandm": bandm,
            "sel": sel,
            "gb": gbj,
        })
    return in_maps


def kernel(x, w_qkv, wo, lambda_q1, lambda_q2, lambda_k1, lambda_k2,
           gamma, beta, _trace=False):
    nc = _get_nc()
    in_maps = _host_prep(x, w_qkv, wo, lambda_q1, lambda_q2, lambda_k1,
                         lambda_k2, gamma, beta)
    res = run_bass_kernel_spmd(nc, in_maps, list(range(NC)), trace=_trace)
    rows = np.concatenate([res.results[j]["out"] for j in range(NC)], axis=0)
    out = rows.reshape(B, S, E).astype(np.float32)
    if _trace:
        kernel.last_results = res
    return out


# revision 33
# speedup vs baseline: 1.9639x; 1.9639x over previous
"""Differential attention + quirky GroupNorm + output proj on 8 TRN2 NeuronCores.

Strategy (head-sharded attention, token-sharded norm+proj):
  - Host preps transposed layouts (xT, per-core wqkvT slice, woT) and RoPE
    cos/sin tables. External inputs stay fp32 (bf16 I/O corrupts via
    PJRT); weights/activations are converted to bf16 on device so every big
    matmul runs 1-pass bf16 with FWL weight loads.
  - Each core computes QKV for its 2 heads (contraction over full E), applies
    RoPE, runs the two differential softmaxes (full S x S, float +1.0 mask
    above the diagonal - NOT causal). The two 64-dim score contractions are
    issued as row-disjoint concurrent matmuls (rows 0-63 / 64-127 of the PE
    array), and PV accumulation is inlined in the same kt loop.
  - Softmax normalization: row sums via ones-matmul, fast approx reciprocal,
    then a K=1 broadcast matmul per half folds 1/sum (and -lambda) into
    [128,512]; both halves combine into the bf16 A2A payload.
  - Device AllToAll (bf16 payload) redistributes A from head-sharded to
    token-sharded (512 tokens per core; the quirky reshape makes GroupNorm
    groups equal 128-token blocks x all channels, so groups stay core-local).
  - Each core computes GroupNorm stats for its 4 groups, then output rows
    out[t, :] = 0.2*(gamma[t]*Ahat[t,:] + beta[t]) @ woT with the norm scale
    folded into the PSUM evacuation and the beta/mean terms folded into one
    rank-1 row term. wo is converted to bf16 on device.
"""

import math

import numpy as np

import concourse.bass as bass
import concourse.tile as tile
from concourse import bacc, mybir
from concourse.bass_utils import run_bass_kernel_spmd

F32 = mybir.dt.float32
F32R = mybir.dt.float32r
BF16 = mybir.dt.bfloat16
AX = mybir.AxisListType
OP = mybir.AluOpType
ACTF = mybir.ActivationFunctionType

B, S, E, H = 2, 2048, 2048, 16
HD = E // H                # 128
NC = 8                     # cores
HPC = H // NC              # 2 heads per core
CPC = HPC * HD             # 256 channels per core
T = B * S                  # 4096 tokens
TPC = T // NC              # 512 tokens per core
NG = TPC // HD             # 4 groups per core (128-token blocks)
GROUP_N = float(HD * E)    # 262144 elements per group
INIT_LAMBDA = 0.8
EPS = 1e-5
SCALER = HD ** -0.5


def _mm(x):
    return x.bitcast(F32R)


def _attn_bh(nc, pools, consts, h, b):
    """Differential attention for one (batch, local-head) pair."""
    vp, pp, s2tmp, ps_sc, ps_sum, ps_o, ps_bc = pools
    q_sb, k_sb, masks_sb, ones_col_bf, rows_sc, v_dram, a2a_in = consts
    bh = b * HPC + h
    v_bh = vp.tile([128, 16 * HD], BF16, tag="v_bh")
    nc.sync.dma_start(
        v_bh[:].rearrange("p (kt d) -> p kt d", kt=16),
        v_dram[b, :, h * HD:(h + 1) * HD].rearrange("(kt p) d -> p kt d", p=128),
    )
    for qc in range(4):
        sum0 = ps_sum.tile([1, 512], F32, tag="sum0")
        sum1 = ps_sum.tile([1, 512], F32, tag="sum1")
        sums = [sum0, sum1]
        op0 = ps_o.tile([128, 512], F32, tag="o0")
        op1 = ps_o.tile([128, 512], F32, tag="o1")
        ops = [op0, op1]
        for kt in range(16):
            scs = []
            for half in range(2):
                hs = half * 64
                sc_ps = ps_sc.tile([128, 512], F32, tag="sc")
                nc.tensor.matmul(
                    sc_ps[:],
                    k_sb[hs:hs + 64,
                         bh * S + kt * 128:bh * S + kt * 128 + 128],
                    q_sb[hs:hs + 64,
                         bh * S + qc * 512:bh * S + (qc + 1) * 512],
                    start=True, stop=True,
                )
                scs.append(sc_ps)
            o = kt - 4 * qc
            for half in range(2):
                pt = pp.tile([128, 512], BF16, tag="p")
                if 0 <= o < 4:
                    scr = s2tmp.tile([128, 512], F32, tag="bandscr")
                    nc.vector.scalar_tensor_tensor(
                        out=scr[:], in0=scs[half][:], scalar=SCALER,
                        in1=masks_sb[:, o * 512:(o + 1) * 512],
                        op0=OP.mult, op1=OP.add,
                    )
                    nc.scalar.activation(pt[:], scr[:], ACTF.Exp)
                else:
                    bias = 1.0 if o >= 4 else 0.0
                    nc.scalar.activation(pt[:], scs[half][:], ACTF.Exp,
                                         bias=bias, scale=SCALER)
                nc.tensor.matmul(
                    sums[half][:], ones_col_bf[:], pt[:],
                    start=(kt == 0), stop=(kt == 15),
                )
                nc.tensor.matmul(
                    ops[half][:],
                    v_bh[:, kt * HD:(kt + 1) * HD], pt[:],
                    start=(kt == 0), stop=(kt == 15),
                )
        # fast approx 1/sum per half, straight from PSUM
        recip0 = s2tmp.tile([1, 512], F32, tag="recip0")
        recip1 = s2tmp.tile([1, 512], F32, tag="recip1")
        rbf0 = s2tmp.tile([1, 512], BF16, tag="rbf0")
        rbf1 = s2tmp.tile([1, 512], BF16, tag="rbf1")
        recips = [recip0, recip1]
        rbfs = [rbf0, rbf1]
        for half in range(2):
            nc.vector.reciprocal_approx_fast(
                out=recips[half][:], in_=sums[half][:],
            )
            nc.vector.tensor_copy(rbfs[half][:], recips[half][:])
        t_half = [None, None]
        for half in range(2):
            bc_ps = ps_bc.tile([128, 512], F32, tag="bc")
            nc.tensor.matmul(
                bc_ps[:], rows_sc[half][:], rbfs[half][:],
                start=True, stop=True,
            )
            bcs = s2tmp.tile([128, 512], F32, tag=f"bcs{half}")
            nc.scalar.copy(bcs[:], bc_ps[:])
            th = s2tmp.tile([128, 512], F32, tag=f"th{half}")
            nc.vector.tensor_tensor(th[:], ops[half][:], bcs[:], OP.mult)
            t_half[half] = th
        a_sb = s2tmp.tile([128, 512], BF16, tag="a_sb")
        nc.vector.tensor_tensor(a_sb[:], t_half[0][:], t_half[1][:], OP.add)
        dest = b * 4 + qc
        nc.sync.dma_start(a2a_in[h][dest, :, :], a_sb[:])


def build_nc():
    nc = bacc.Bacc("TRN2", target_bir_lowering=False, debug=False, num_devices=NC)

    xT = nc.declare_dram_parameter("xT", [E, T], F32, isOutput=False)
    wqkvT = nc.declare_dram_parameter("wqkvT", [E, 3 * CPC], F32, isOutput=False)
    woT = nc.declare_dram_parameter("woT", [E, E], F32, isOutput=False)
    cosd = nc.declare_dram_parameter("cosd", [HD, S], F32, isOutput=False)
    sind = nc.declare_dram_parameter("sind", [HD, S], F32, isOutput=False)
    bandm = nc.declare_dram_parameter("bandm", [4, 128, 512], F32, isOutput=False)
    lam_a = nc.declare_dram_parameter("lam_a", [2, HD], F32, isOutput=False)
    lam_b = nc.declare_dram_parameter("lam_b", [2, HD], F32, isOutput=False)
    sgn2 = nc.declare_dram_parameter("sgn2", [2, 1], F32, isOutput=False)
    ones128 = nc.declare_dram_parameter("ones128", [1, 128], F32, isOutput=False)
    gb = nc.declare_dram_parameter("gb", [2, TPC], F32, isOutput=False)
    out = nc.declare_dram_parameter("out", [TPC, E], F32, isOutput=True)

    # internal DRAM
    v_dram = nc.dram_tensor("v_dram", [B, S, CPC], BF16)
    a2a_in = [nc.dram_tensor(f"a2a_in{h}", [NC, HD, TPC], BF16)
              for h in range(HPC)]
    a2a_out = [nc.dram_tensor(f"a2a_out{h}", [NC, HD, TPC], BF16)
               for h in range(HPC)]

    with tile.TileContext(nc) as tc, \
         nc.allow_low_precision(reason="bf16 matmuls; tolerance 2e-2"):
        with tc.tile_pool(name="small", bufs=1) as small:
            ones_col = small.tile([128, 1], F32, tag="ones_col")
            nc.sync.dma_start(
                ones_col[:].bitcast(F32R),
                ones128[0, :].rearrange("(p o) -> p o", o=1).bitcast(F32R),
            )
            ones_row = small.tile([1, 128], F32, tag="ones_row")
            nc.sync.dma_start(ones_row[:].bitcast(F32R),
                              ones128[:, :].bitcast(F32R))
            ones_col_bf = small.tile([128, 1], BF16, tag="ones_col_bf")
            nc.vector.tensor_copy(ones_col_bf[:], ones_col[:])

            # ---- lambda scalar ----
            la = small.tile([2, HD], F32, tag="la")
            nc.sync.dma_start(la[:], lam_a[:, :])
            lb = small.tile([2, HD], F32, tag="lb")
            nc.sync.dma_start(lb[:], lam_b[:, :])
            prod = small.tile([2, HD], F32, tag="lprod")
            nc.vector.tensor_tensor(prod[:], la[:], lb[:], OP.mult)
            dots = small.tile([2, 1], F32, tag="ldots")
            nc.vector.tensor_reduce(
                dots[:], prod[:].rearrange("p (n u) -> p n u", u=HD), AX.X, OP.add
            )
            lexp = small.tile([2, 1], F32, tag="lexp")
            nc.scalar.activation(lexp[:], dots[:], ACTF.Exp)
            sv = small.tile([2, 1], F32, tag="sv")
            nc.sync.dma_start(sv[:], sgn2[:, :])
            with tc.tile_pool(name="ps_lam", bufs=1, space="PSUM") as ps_lam:
                lam_ps = ps_lam.tile([1, 1], F32, tag="lam_ps")
                nc.tensor.matmul(lam_ps[:], sv[:], lexp[:],
                                 start=True, stop=True)
                lam_t = small.tile([1, 1], F32, tag="lam_t")
                nc.vector.tensor_scalar(
                    out=lam_t[:], in0=lam_ps[:], scalar1=INIT_LAMBDA,
                    scalar2=None, op0=OP.add,
                )
            neglam_row = small.tile([1, 128], F32, tag="neglam_row")
            nc.vector.tensor_scalar(
                out=neglam_row[:], in0=ones_row[:],
                scalar1=lam_t[0:1, 0:1],
                scalar2=-1.0, op0=OP.mult, op1=OP.mult,
            )
            ones_row_bf = small.tile([1, 128], BF16, tag="ones_row_bf")
            nc.vector.tensor_copy(ones_row_bf[:], ones_row[:])
            neglam_row_bf = small.tile([1, 128], BF16, tag="neglam_row_bf")
            nc.vector.tensor_copy(neglam_row_bf[:], neglam_row[:])
            rows_sc = (ones_row_bf, neglam_row_bf)

            with tc.tile_pool(name="qkres", bufs=1) as qkres:
                q_sb = qkres.tile([128, 2 * HPC * S], BF16, tag="q_sb")
                k_sb = qkres.tile([128, 2 * HPC * S], BF16, tag="k_sb")

                # ======== stage 1: QKV + RoPE (bf16 matmuls) ========
                with tc.tile_pool(name="s1res", bufs=1) as s1res, \
                     tc.tile_pool(name="wstg", bufs=3) as wstg, \
                     tc.tile_pool(name="xtp", bufs=6) as xtp, \
                     tc.tile_pool(name="xbfp", bufs=20) as xbfp, \
                     tc.tile_pool(name="s1tmp", bufs=6) as s1tmp, \
                     tc.tile_pool(name="vtmp", bufs=4) as vtmp, \
                     tc.tile_pool(name="ps_qk", bufs=5, space="PSUM") as ps_qk, \
                     tc.tile_pool(name="ps_v", bufs=3, space="PSUM") as ps_v:
                    w_sb = s1res.tile([128, 16 * 3 * CPC], BF16, tag="w_sb")
                    for et in range(16):
                        wst = wstg.tile([128, 3 * CPC], F32, tag="wst")
                        nc.sync.dma_start(
                            wst[:].bitcast(F32R),
                            wqkvT[et * 128:(et + 1) * 128, :].bitcast(F32R),
                        )
                        nc.vector.tensor_copy(
                            w_sb[:, et * 3 * CPC:(et + 1) * 3 * CPC], wst[:]
                        )
                    cos_sb = s1res.tile([HD, S], F32, tag="cos_sb")
                    nc.sync.dma_start(cos_sb[:], cosd[:, :])
                    sin_sb = s1res.tile([HD, S], F32, tag="sin_sb")
                    nc.sync.dma_start(sin_sb[:], sind[:, :])

                    for tci in range(T // 512):
                        b = tci // 4
                        sc = tci % 4
                        xbfs = []
                        for et in range(16):
                            xt = xtp.tile([128, 512], F32, tag="xt")
                            nc.sync.dma_start(
                                xt[:].bitcast(F32R),
                                xT[et * 128:(et + 1) * 128,
                                   tci * 512:(tci + 1) * 512].bitcast(F32R),
                            )
                            xbf = xbfp.tile([128, 512], BF16, tag="xbf")
                            nc.vector.tensor_copy(xbf[:], xt[:])
                            xbfs.append(xbf)
                        # q, k channel-major [hd, 512 tokens] per local head
                        for which, dst in ((0, q_sb), (1, k_sb)):
                            for h in range(HPC):
                                ps = ps_qk.tile([128, 512], F32, tag="psqk")
                                for et in range(16):
                                    wcol = et * 3 * CPC + which * CPC + h * HD
                                    nc.tensor.matmul(
                                        ps[:],
                                        w_sb[:, wcol:wcol + HD],
                                        xbfs[et][:],
                                        start=(et == 0), stop=(et == 15),
                                    )
                                # RoPE: dst = ps*cos + rotperm(ps)*sinsgn
                                csl = cos_sb[:, sc * 512:(sc + 1) * 512]
                                ssl = sin_sb[:, sc * 512:(sc + 1) * 512]
                                qc = s1tmp.tile([128, 512], F32, tag="ropeqc")
                                nc.vector.tensor_tensor(qc[:], ps[:], csl, OP.mult)
                                rot = s1tmp.tile([128, 512], F32, tag="roperot")
                                nc.scalar.copy(rot[0:64, :], ps[64:128, :])
                                nc.scalar.copy(rot[64:128, :], ps[0:64, :])
                                nc.vector.tensor_tensor(rot[:], rot[:], ssl, OP.mult)
                                col = (b * HPC + h) * S + sc * 512
                                nc.vector.tensor_tensor(
                                    dst[:, col:col + 512], qc[:], rot[:], OP.add
                                )
                        # v token-major [t, 256]
                        for ts4 in range(4):
                            ps = ps_v.tile([128, CPC], F32, tag="psv")
                            for et in range(16):
                                wcol = et * 3 * CPC + 2 * CPC
                                nc.tensor.matmul(
                                    ps[:],
                                    xbfs[et][:, ts4 * 128:(ts4 + 1) * 128],
                                    w_sb[:, wcol:wcol + CPC],
                                    start=(et == 0), stop=(et == 15),
                                )
                            vsb = vtmp.tile([128, CPC], BF16, tag="vsb")
                            nc.scalar.copy(vsb[:], ps[:])
                            trow = sc * 512 + ts4 * 128
                            nc.sync.dma_start(
                                v_dram[b, trow:trow + 128, :], vsb[:]
                            )

                # ======== stage 2: differential attention ========
                with tc.tile_pool(name="s2res", bufs=1) as s2res, \
                     tc.tile_pool(name="vp", bufs=2) as vp, \
                     tc.tile_pool(name="pp", bufs=6) as pp, \
                     tc.tile_pool(name="s2tmp", bufs=2) as s2tmp, \
                     tc.tile_pool(name="s4res", bufs=1) as s4res, \
                     tc.tile_pool(name="wstg4", bufs=2) as wstg4, \
                     tc.tile_pool(name="wop", bufs=16) as wop, \
                     tc.tile_pool(name="s4tmp", bufs=4) as s4tmp, \
                     tc.tile_pool(name="sqscratch", bufs=2) as sqp:
                    with tc.tile_pool(name="ps_sc", bufs=3, space="PSUM") as ps_sc, \
                         tc.tile_pool(name="ps_sum", bufs=1, space="PSUM") as ps_sum, \
                         tc.tile_pool(name="ps_o", bufs=1, space="PSUM") as ps_o, \
                         tc.tile_pool(name="ps_bc", bufs=1, space="PSUM") as ps_bc:
                        masks_sb = s2res.tile([128, 4 * 512], F32, tag="masks_sb")
                        for o in range(4):
                            nc.sync.dma_start(
                                masks_sb[:, o * 512:(o + 1) * 512], bandm[o, :, :]
                            )
                        pools = (vp, pp, s2tmp, ps_sc, ps_sum, ps_o, ps_bc)
                        consts = (q_sb, k_sb, masks_sb, ones_col_bf, rows_sc,
                                  v_dram, a2a_in)
                        for h in range(HPC):
                            for b in range(B):
                                _attn_bh(nc, pools, consts, h, b)
                            nc.gpsimd.collective_compute(
                                "AllToAll",
                                OP.bypass,
                                replica_groups=[list(range(NC))],
                                ins=[a2a_in[h].ap().opt()],
                                outs=[a2a_out[h].ap().opt()],
                            )

                        # G loads + per-tile stat partials: depend only on the
                        # per-head collectives, so the scheduler can overlap
                        # them with remaining attention.
                        g_sb = s4res.tile([128, 16 * TPC], BF16, tag="g_sb")
                        st_sum = s4res.tile([128, NG, 16], F32, tag="st_sum")
                        st_sq = s4res.tile([128, NG * 16], F32, tag="st_sq")
                        for ct in range(16):
                            nc.sync.dma_start(
                                g_sb[:, ct * TPC:(ct + 1) * TPC],
                                a2a_out[ct % 2][ct // 2, :, :],
                            )
                            gt = g_sb[:, ct * TPC:(ct + 1) * TPC]
                            nc.vector.tensor_reduce(
                                st_sum[:, :, ct],
                                gt.rearrange("p (g n) -> p g n", g=NG),
                                AX.X, OP.add,
                            )
                            for g in range(NG):
                                scr = sqp.tile([128, HD], F32, tag="sqs")
                                nc.scalar.activation(
                                    scr[:], gt[:, g * HD:(g + 1) * HD],
                                    ACTF.Square,
                                    accum_out=st_sq[:, g * 16 + ct:
                                                    g * 16 + ct + 1],
                                )

                    # ======== stage 4: stats finalize + output projection ====
                    with tc.tile_pool(name="ps_st", bufs=1, space="PSUM") as ps_st, \
                         tc.tile_pool(name="ps_rs", bufs=1, space="PSUM") as ps_rs, \
                         tc.tile_pool(name="ps_b", bufs=1, space="PSUM") as ps_b, \
                         tc.tile_pool(name="ps_p", bufs=4, space="PSUM") as ps_p:
                        red8 = s4res.tile([128, 2 * NG], F32, tag="red8")
                        nc.vector.tensor_reduce(
                            red8[:, 0:NG].bitcast(F32R), st_sum[:],
                            AX.X, OP.add
                        )
                        nc.vector.tensor_reduce(
                            red8[:, NG:2 * NG].bitcast(F32R),
                            st_sq[:].rearrange("p (g c) -> p g c", g=NG),
                            AX.X, OP.add
                        )
                        stat_ps = ps_st.tile([1, 2 * NG], F32, tag="statp")
                        nc.tensor.matmul(
                            stat_ps[:], _mm(ones_col[:]), _mm(red8[:]),
                            start=True, stop=True,
                        )
                        srow = s4res.tile([1, 2 * NG], F32, tag="srow")
                        nc.vector.tensor_scalar(
                            out=srow[:], in0=stat_ps[:], scalar1=1.0 / GROUP_N,
                            scalar2=None, op0=OP.mult,
                        )  # [mean_g | E[x^2]_g]
                        var_r = s4res.tile([1, NG], F32, tag="var_r")
                        m2 = s4res.tile([1, NG], F32, tag="m2")
                        nc.vector.tensor_tensor(
                            m2[:], srow[:, 0:NG], srow[:, 0:NG], OP.mult
                        )
                        nc.vector.tensor_tensor(
                            var_r[:], srow[:, NG:2 * NG], m2[:], OP.subtract
                        )
                        eps_t = s4res.tile([1, 1], F32, tag="eps_t")
                        nc.gpsimd.memset(eps_t[:], EPS)
                        std_r = s4res.tile([1, NG], F32, tag="std_r")
                        nc.scalar.activation(std_r[:], var_r[:], ACTF.Sqrt,
                                             bias=eps_t[:])
                        ab_row = s4res.tile([1, 2 * NG], F32, tag="ab_row")
                        nc.vector.reciprocal(ab_row[:, 0:NG].bitcast(F32R),
                                             std_r[:])
                        mtmp = s4res.tile([1, NG], F32, tag="mtmp")
                        nc.vector.tensor_tensor(
                            mtmp[:], srow[:, 0:NG], ab_row[:, 0:NG], OP.mult
                        )
                        nc.vector.tensor_scalar(
                            out=ab_row[:, NG:2 * NG].bitcast(F32R), in0=mtmp[:],
                            scalar1=-1.0, scalar2=None, op0=OP.mult,
                        )  # b_g = -mean*rstd
                        ab_ps = ps_st.tile([128, 2 * NG], F32, tag="abp")
                        nc.tensor.matmul(
                            ab_ps[:], _mm(ones_row[:]), _mm(ab_row[:]),
                            start=True, stop=True,
                        )
                        ab_bc = s4res.tile([128, 2 * NG], F32, tag="ab_bc")
                        nc.scalar.copy(ab_bc[:], ab_ps[:])

                        gamma_col = s4res.tile([128, NG], F32, tag="gamma_col")
                        nc.sync.dma_start(
                            gamma_col[:], gb[0, :].rearrange("(c p) -> p c", p=128)
                        )
                        m1_col = s4res.tile([128, NG], F32, tag="m1_col")
                        nc.vector.scalar_tensor_tensor(
                            out=m1_col[:], in0=gamma_col[:],
                            scalar=(1.0 - INIT_LAMBDA), in1=ab_bc[:, 0:NG],
                            op0=OP.mult, op1=OP.mult,
                        )
                        gamma_row = s4res.tile([1, TPC], F32, tag="gamma_row")
                        nc.sync.dma_start(gamma_row[:], gb[0:1, :])
                        beta_row = s4res.tile([1, TPC], F32, tag="beta_row")
                        nc.sync.dma_start(beta_row[:], gb[1:2, :])
                        m2_row = s4res.tile([1, TPC], F32, tag="m2_row")
                        m2tmp = s4res.tile([1, TPC], F32, tag="m2tmp")
                        nc.vector.tensor_tensor(
                            m2tmp[:].rearrange("o (g u) -> o g u", g=NG),
                            gamma_row[:].rearrange("o (g u) -> o g u", g=NG),
                            ab_row[:, NG:2 * NG].rearrange(
                                "o (g u) -> o g u", u=1
                            ).to_broadcast([1, NG, HD]),
                            OP.mult,
                        )
                        nc.vector.tensor_tensor(
                            m2tmp[:], m2tmp[:], beta_row[:], OP.add
                        )
                        nc.vector.tensor_scalar(
                            out=m2_row[:].bitcast(F32R), in0=m2tmp[:],
                            scalar1=(1.0 - INIT_LAMBDA),
                            scalar2=None, op0=OP.mult,
                        )

                        rowsum = s4res.tile([1, E], F32, tag="rowsum")
                        for oc in range(4):
                            wots = []
                            for ct in range(16):
                                wst = wstg4.tile([128, 512], F32, tag="wst4")
                                nc.sync.dma_start(
                                    wst[:].bitcast(F32R),
                                    woT[ct * 128:(ct + 1) * 128,
                                        oc * 512:(oc + 1) * 512].bitcast(F32R),
                                )
                                wot = wop.tile([128, 512], BF16, tag="wot")
                                nc.vector.tensor_copy(wot[:], wst[:])
                                wots.append(wot)
                            rs_ps = ps_rs.tile([1, 512], F32, tag="rs")
                            for ct in range(16):
                                nc.tensor.matmul(
                                    rs_ps[:], ones_col_bf[:], wots[ct][:],
                                    start=(ct == 0), stop=(ct == 15),
                                )
                            nc.scalar.copy(
                                rowsum[:, oc * 512:(oc + 1) * 512].bitcast(F32R),
                                rs_ps[:],
                            )
                            for tcg in range(NG):
                                po = ps_p.tile([128, 512], F32, tag="po")
                                for ct in range(16):
                                    nc.tensor.matmul(
                                        po[:],
                                        g_sb[:, ct * TPC + tcg * 128:
                                             ct * TPC + (tcg + 1) * 128],
                                        wots[ct][:],
                                        start=(ct == 0), stop=(ct == 15),
                                    )
                                bps = ps_b.tile([128, 512], F32, tag="bps")
                                nc.tensor.matmul(
                                    bps[:],
                                    _mm(m2_row[0:1, tcg * 128:(tcg + 1) * 128]),
                                    _mm(rowsum[0:1, oc * 512:(oc + 1) * 512]),
                                    start=True, stop=True,
                                )
                                osb = s4tmp.tile([128, 512], F32, tag="osb")
                                nc.scalar.activation(
                                    osb[:], po[:], ACTF.Copy,
                                    scale=m1_col[:, tcg:tcg + 1],
                                )
                                nc.vector.tensor_tensor(osb[:], osb[:], bps[:],
                                                        OP.add)
                                nc.sync.dma_start(
                                    out[tcg * 128:(tcg + 1) * 128,
                                        oc * 512:(oc + 1) * 512],
                                    osb[:],
                                )

    nc.compile()
    return nc


_NC_CACHE = None


def _get_nc():
    global _NC_CACHE
    if _NC_CACHE is None:
        _NC_CACHE = build_nc()
    return _NC_CACHE


def _host_prep(x, w_qkv, wo, lambda_q1, lambda_q2, lambda_k1, lambda_k2,
               gamma, beta):
    x = np.asarray(x, dtype=np.float32)
    w_qkv = np.asarray(w_qkv, dtype=np.float32)
    wo = np.asarray(wo, dtype=np.float32)
    gamma = np.asarray(gamma, dtype=np.float32)
    beta = np.asarray(beta, dtype=np.float32)

    xT = np.ascontiguousarray(x.reshape(T, E).T)
    woT = np.ascontiguousarray(wo.T)

    # RoPE tables, channel-major with sign folded into sin
    inv = 1.0 / (10000.0 ** (np.arange(0, HD, 2, dtype=np.float32) / HD))
    ang = np.arange(S, dtype=np.float32)[:, None] * inv[None, :]  # (S, 64)
    ang = np.concatenate([ang, ang], axis=-1)                     # (S, 128)
    cosd = np.ascontiguousarray(np.cos(ang).T.astype(np.float32))  # (128, S)
    sin_t = np.sin(ang).T.astype(np.float32)
    sind = np.ascontiguousarray(
        np.concatenate([-sin_t[:64], sin_t[64:]], axis=0)
    )

    # band masks: mask_o[ki, qi] = 1.0 iff (o*128 + ki) > qi
    o_idx = np.arange(4)[:, None, None] * 128
    ki = np.arange(128)[None, :, None]
    qi = np.arange(512)[None, None, :]
    bandm = ((o_idx + ki) > qi).astype(np.float32)

    lam_a = np.ascontiguousarray(
        np.stack([lambda_q1, lambda_q2]).astype(np.float32)
    )
    lam_b = np.ascontiguousarray(
        np.stack([lambda_k1, lambda_k2]).astype(np.float32)
    )

    in_maps = []
    for j in range(NC):
        h0 = HPC * j
        rows_q = w_qkv[h0 * HD:(h0 + HPC) * HD, :]
        rows_k = w_qkv[E + h0 * HD:E + (h0 + HPC) * HD, :]
        rows_v = w_qkv[2 * E + h0 * HD:2 * E + (h0 + HPC) * HD, :]
        wqkvT = np.ascontiguousarray(
            np.concatenate([rows_q.T, rows_k.T, rows_v.T], axis=1)
            .astype(np.float32)
        )
        sl = (j % 4) * TPC
        gbj = np.ascontiguousarray(
            np.stack([gamma[sl:sl + TPC], beta[sl:sl + TPC]])
        )
        in_maps.append({
            "sgn2": np.array([[1.0], [-1.0]], dtype=np.float32),
            "ones128": np.ones((1, 128), dtype=np.float32),
            "xT": xT,
            "wqkvT": wqkvT,
            "woT": woT,
            "cosd": cosd,
            "sind": sind,
            "bandm": bandm,
            "lam_a": lam_a,
            "lam_b": lam_b,
            "gb": gbj,
        })
    return in_maps


def kernel(x, w_qkv, wo, lambda_q1, lambda_q2, lambda_k1, lambda_k2,
           gamma, beta, _trace=False):
    nc = _get_nc()
    in_maps = _host_prep(x, w_qkv, wo, lambda_q1, lambda_q2, lambda_k1,
                         lambda_k2, gamma, beta)
    res = run_bass_kernel_spmd(nc, in_maps, list(range(NC)), trace=_trace)
    rows = np.concatenate([res.results[j]["out"] for j in range(NC)], axis=0)
    out = rows.reshape(B, S, E).astype(np.float32)
    if _trace:
        kernel.last_results = res
    return out
